# revision 26
# baseline (speedup 1.0000x reference)
"""Trainium2 Bass kernel for nn_DeformableConvLSTMCell_33895881900284.

Full (unsharded) inputs in, full outputs out. Data-parallel over batch across
8 NeuronCores (8 batches per core), conv weights / gate params replicated.

Math per the reference:
  outI  = conv3x3_same(inputs, wconvInput)
  g     = tanh(outI + conv3x3_same(hidden_prev, wconvHidden) + gateBias)
  gapI  = mean_hw(outI);  gapH = mean_hw(hidden_prev)          # [B, D]
  i/f/o = sigmoid(wx*gapI + wh*gapH + bias)                    # [B, D]
  tiled gate: value used at (b, h, w, c) is gate[(28*b + h) % 64, c]
  state  = f*state_prev + i*g;  hidden = o*tanh(state)

Design (PE-bound; conv is ~95% of PE work):
  - conv runs in fp8(e4m3) DoubleRow mode: each matmul contracts K=256 (both
    128-channel chunks) at 0.5 cycles/output-column.  Accuracy is recovered
    with a 3-term residual scheme, all power-of-2 scaled so one PSUM
    accumulation holds everything:
        psum = x_hi*(W* + QR) + XR*WB + 256*gateBias
    where W* = q8(256 w), QR = q8(256 w - W*), x_hi = q8(x),
    XR = q8(32 (x - x_hi)), WB = q8(8 w); then g = tanh(psum / 256).
    (measured rel-err ~1.2e-2 vs the 2e-2 gate; plain fp8 is 0.12)
  - activations live channel-major on a padded 30x30 "30-grid" so every tap's
    moving operand is one contiguous 1-D slice (3-D AP as DoubleRow needs);
    output windows are nr rows x 30 cols with 2 junk columns per row.
  - gateBias is injected into PSUM by a bf16/f16 identity-stationary matmul
    (moving = channel-major gateBias slice), so no vector-engine bias add;
    ACT applies tanh with scale=1/256 straight from PSUM.
  - GAP path: masked pixel sums of the raw fp8 inputs ride tiny matmuls
    (stationary = natural-layout tile, moving = signed 0/+-1 masks, 16 output
    cols), are AllGather'ed (raw, unnormalized), and every core rebuilds the
    gates for all 64 batches locally: 18 small matmuls (gapI tap-combos by
    linearity of conv+GAP), tensor_scalar gate algebra, sigmoid, a PE
    transpose, and a per-core selection-matrix matmul that realizes the
    (28 b + h) % 64 gate scrambling with plain data (SPMD-safe).
  - elementwise runs column-major (pixel = 28*w + h) in f16 so the per-row
    gate broadcast is a middle-axis stride-0 AP and DVE gets its 2x mode for
    the add; outputs are stored f16 column-major and the host restores NHWC.
"""
import numpy as np

import bass_rust
import concourse.bass as bass
import concourse.mybir as mybir
import concourse.tile as tile
from concourse.bass_utils import run_bass_kernel_spmd

F32 = mybir.dt.float32
F16 = mybir.dt.float16
FP8 = mybir.dt.float8e4
AF = mybir.ActivationFunctionType
ALU = mybir.AluOpType
DR = mybir.MatmulPerfMode.DoubleRow

N_CORES = 8
B, H, W, CIN, D = 64, 28, 28, 256, 256
BL = B // N_CORES
PIX = H * W                    # 784
G30 = 912                      # padded 30-grid row-major length (30*30 -> 912)
SW = 256.0                     # weight scale
SXR = 32.0                     # activation-residual scale
WINDOWS = [(0, 8), (8, 8), (16, 8), (24, 4)]   # (r0, nr)
TAPS = [(kh, kw) for kh in range(3) for kw in range(3)]
NGRP = 9                       # gapI mask groups
MCOL = 16                      # mask columns (9 gapI + 1 full-sum + pad)

# gapI tap combos per mask group (group signs live in the masks)
GAP_TAPSETS = [list(range(9)), [6, 7, 8], [0, 1, 2], [2, 5, 8], [0, 3, 6],
               [8], [6], [2], [0]]

# ---------------------------------------------------------------------------
# walrus fixup: split semaphore waits that exceed the per-instruction budget
MAX_WAITS = 1


def _split_excess_sem_waits(nc):
    counter = [0]
    for fn in nc.m.functions:
        for bb in fn.blocks:
            insts = bb.instructions
            i = 0
            while i < len(insts):
                inst = insts[i]
                si = inst.sync_info
                if si is not None and si.on_wait and len(si.on_wait) > MAX_WAITS:
                    waits = list(si.on_wait)
                    excess = waits[:-MAX_WAITS]
                    keep = waits[-MAX_WAITS:]
                    new_insts = []
                    for j in range(0, len(excess), MAX_WAITS):
                        chunk = excess[j:j + MAX_WAITS]
                        noop = mybir.InstNoOp(
                            name=f"I-waitsplit-{counter[0]}", ins=[], outs=[])
                        counter[0] += 1
                        noop.engine = inst.engine
                        noop.sync_info = bass_rust.SyncInfo(
                            on_wait=chunk, on_update=[])
                        nc.register_instruction(noop)
                        new_insts.append(noop)
                    inst.sync_info = bass_rust.SyncInfo(
                        on_wait=keep, on_update=list(si.on_update))
                    insts[i:i] = new_insts
                    i += len(new_insts)
                i += 1
    return nc


# ---------------------------------------------------------------------------
def build_nc():
    nc = bass.Bass("TRN2", target_bir_lowering=False, debug=False,
                   num_devices=N_CORES)

    dram = {}
    # channel-major padded 30-grid fp8 activations (hi + scaled residual)
    for nm in ("xinh", "xinr", "xhdh", "xhdr"):
        dram[nm] = nc.dram_tensor(nm, [BL, 2, 128, G30], FP8,
                                  kind="ExternalInput").ap()
    # state_prev, column-major f16
    dram["sp"] = nc.dram_tensor("sp", [BL, 2, 128, PIX], F16,
                                kind="ExternalInput").ap()
    # natural-layout fp8 copies for the GAP masked sums (0=inputs, 1=hidden)
    dram["nat"] = nc.dram_tensor("nat", [BL, 2, 112, 7 * 256], FP8,
                                 kind="ExternalInput").ap()
    # fp8 conv weight packs: (term, conv, tap, dc) blocks of [128, 2, 128]
    dram["wpk"] = nc.dram_tensor("wpk", [128, 3 * 2 * 9 * 2 * 256], FP8,
                                 kind="ExternalInput").ap()
    # channel-major 30-grid gateBias, pre-scaled by SW, fp8 hi+lo planes
    dram["gb30"] = nc.dram_tensor("gb30", [2, 2, 128, G30], FP8,
                                  kind="ExternalInput").ap()
    # gapI tap-combo weights (already / 784), f16: (cc, g) blocks [128, 256]
    dram["gapw"] = nc.dram_tensor("gapw", [128, 2 * NGRP * 256], F16,
                                  kind="ExternalInput").ap()
    # signed GAP masks
    dram["masks"] = nc.dram_tensor("masks", [112, 7 * MCOL], FP8,
                                   kind="ExternalInput").ap()
    # per-core gate selection matrix
    dram["smat"] = nc.dram_tensor("smat", [64, BL * H], F16,
                                  kind="ExternalInput").ap()
    # gate vectors: per dc 9 cols (wxi, whi/784, ib, wxf, whf/784, fb, ...)
    dram["vecs"] = nc.dram_tensor("vecs", [128, 2, 9], F32,
                                  kind="ExternalInput").ap()
    # outputs, column-major f16
    dram["st"] = nc.dram_tensor("st_out", [BL, 2, 128, PIX], F16,
                                kind="ExternalOutput").ap()
    dram["hd"] = nc.dram_tensor("hd_out", [BL, 2, 128, PIX], F16,
                                kind="ExternalOutput").ap()
    # collective buffers (raw GAP sums)
    dram["cc_in"] = nc.dram_tensor("cc_in", [256, 80], F16,
                                   kind="Internal").ap()
    dram["cc_out"] = nc.dram_tensor("cc_out", [N_CORES * 256, 80], F16,
                                    kind="Internal", addr_space="Shared").ap()

    ctx_mgr = nc.allow_low_precision("fp8 DoubleRow conv + f16 elementwise")
    ctx_mgr.__enter__()
    with tile.TileContext(nc) as tc:
        _build_body(nc, tc, dram)
    ctx_mgr.__exit__(None, None, None)
    return nc


def _build_body(nc, tc, dram):
    from collections import deque
    from contextlib import ExitStack
    ctx = ExitStack()
    pool = lambda **kw: ctx.enter_context(tc.tile_pool(**kw))

    const = pool(name="const", bufs=1)
    wts = pool(name="wts", bufs=1)
    xp = pool(name="xp", bufs=4)         # fp8 activation tiles
    natp = pool(name="natp", bufs=8)     # natural fp8 tiles for GAP
    spp = pool(name="spp", bufs=8)       # state_prev tiles (all resident)
    g30p = pool(name="g30p", bufs=8)     # tanh outputs per (j, dc)
    ewp = pool(name="ewp", bufs=3)       # elementwise temps
    outp = pool(name="outp", bufs=3)     # st/hd staging
    gate = pool(name="gate", bufs=1)     # persistent gate pipeline tiles
    ps_conv = pool(name="ps_conv", bufs=2, space="PSUM")
    ps_ms = pool(name="ps_ms", bufs=2, space="PSUM")
    ps_g = pool(name="ps_g", bufs=2, space="PSUM")
    ps_aux = pool(name="ps_aux", bufs=1, space="PSUM")

    # ---- constants ---------------------------------------------------------
    # identity built on-chip (Pool), copied to f16
    ident = const.tile([128, 128], F32, tag="ident")
    nc.gpsimd.memset(ident[:], 1.0)
    nc.gpsimd.affine_select(ident[:], ident[:], [[-1, 128]],
                            ALU.is_equal, 0.0, base=0, channel_multiplier=1)
    identb = const.tile([128, 128], F16, tag="identb")
    nc.vector.tensor_copy(identb[:], ident[:])

    masks = const.tile([112, 7, MCOL], FP8, tag="masks")
    nc.gpsimd.dma_start(masks[:], dram["masks"].rearrange(
        "p (g m) -> p g m", m=MCOL))

    # conv weights: dc-major packing; dc0 chunks on SP, dc1 on Pool so the
    # Activation engine never runs bulk DMA (its queue must stay clear to
    # drain conv PSUMs promptly)
    wpk = wts.tile([128, 3 * 2 * 9 * 2 * 256], FP8, tag="wpk")
    TCH = 9 * 256

    def load_wpk(r):
        for t12 in r:
            nc.sync.dma_start(wpk[:, t12 * TCH:(t12 + 1) * TCH],
                              dram["wpk"][:, t12 * TCH:(t12 + 1) * TCH])

    load_wpk(range(6))
    # pre-warm the ACT table (tanh/sigmoid share a set); eats the 1.4us
    # table-load before the first conv window needs the engine
    actwarm = const.tile([128, 1], F16, tag="actwarm")
    nc.scalar.activation(actwarm[:], identb[:, 0:1], AF.Tanh)

    gb30 = const.tile([128, 2, 2, G30], FP8, tag="gb30")
    nc.sync.dma_start(gb30[:], dram["gb30"].rearrange("d l p f -> p d l f"))
    load_wpk(range(6, 12))
    identp = const.tile([128, 2, 128], FP8, tag="identp")
    for _l in range(2):
        nc.vector.tensor_copy(identp[:, _l, :], ident[:])

    def wblk(term, conv, t, dc):
        base = (((dc * 3 + term) * 2 + conv) * 9 + t) * 256
        return wpk[:, base:base + 256].rearrange("c (i d) -> c i d", i=2)

    gapw = wts.tile([128, 2, NGRP, 256], F16, tag="gapw")
    nc.sync.dma_start(gapw[:], dram["gapw"].rearrange(
        "c (i g d) -> c i g d", i=2, g=NGRP))
    smat = const.tile([64, BL * H], F16, tag="smat")
    nc.sync.dma_start(smat[:], dram["smat"])
    vecs = const.tile([128, 2, 9], F32, tag="vecs")
    nc.sync.dma_start(vecs[:], dram["vecs"])

    # ---- per-batch loads ---------------------------------------------------
    xt = {}      # (j, nm) -> fp8 tile [128, 2, G30]
    spt = {}     # j -> f16 tile [128, 2, PIX]
    natt = {}    # j -> fp8 tile [112, 2, 7, 256]

    def load_x(j):
        for nm in ("xinh", "xinr", "xhdh", "xhdr"):
            t = xp.tile([128, 2, G30], FP8, tag=nm, name=f"{nm}{j}")
            nc.gpsimd.dma_start(t[:], dram[nm][j].rearrange("i p f -> p i f"))
            xt[(j, nm)] = t

    def load_nat(j):
        t = natp.tile([112, 2, 7, 256], FP8, tag="nat", name=f"nat{j}")
        nc.gpsimd.dma_start(t[:], dram["nat"][j].rearrange(
            "i p (g c) -> p i g c", c=256))
        natt[j] = t

    def load_sp(j):
        t = spp.tile([128, 2, PIX], F16, tag="sp", name=f"sp{j}")
        nc.sync.dma_start(t[:], dram["sp"][j].rearrange("i p f -> p i f"))
        spt[j] = t

    # ---- GAP masked sums ---------------------------------------------------
    raw = [gate.tile([128, 80], F16, tag=f"raw{cc}", name=f"raw{cc}")
           for cc in range(2)]

    def emit_msum(j, tt, cc):
        ps = ps_ms.tile([128, MCOL], F32, tag="ms")
        for g in range(0, 6, 2):
            nc.tensor.matmul(
                ps[:], natt[j][:, tt, g:g + 2, cc * 128:(cc + 1) * 128],
                masks[:, g:g + 2, :], start=(g == 0), stop=False,
                perf_mode=DR)
        nc.tensor.matmul(ps[:], natt[j][:, tt, 6, cc * 128:(cc + 1) * 128],
                         masks[:, 6, :], start=False, stop=True)
        if tt == 0:
            dst = raw[cc][:, 0:72].rearrange("c (g b) -> c g b", b=BL)
            nc.vector.tensor_copy(
                dst[:, :, j:j + 1],
                ps[:, 0:NGRP].rearrange("c (n o) -> c n o", o=1))
        else:
            nc.vector.tensor_copy(raw[cc][:, 72 + j:73 + j],
                                  ps[:, NGRP:NGRP + 1])

    # ---- collective --------------------------------------------------------
    def emit_cc_store_and_allgather():
        for cc in range(2):
            nc.sync.dma_start(dram["cc_in"][cc * 128:(cc + 1) * 128, :],
                              raw[cc][:])
        nc.gpsimd.collective_compute(
            "AllGather", ALU.bypass, replica_groups=[list(range(N_CORES))],
            ins=[dram["cc_in"][:]], outs=[dram["cc_out"][:]])

    # ---- gates for all 64 batches ------------------------------------------
    sel = {}     # (gate_idx, dc) -> [128, BL*H] f16

    def emit_gates():
        rawAllI = [gate.tile([128, NGRP, 64], F16, tag=f"raI{cc}",
                             name=f"raI{cc}") for cc in range(2)]
        rawAllH = [gate.tile([128, 64], F16, tag=f"raH{cc}", name=f"raH{cc}")
                   for cc in range(2)]
        rg = gate.tile([128, 16, 80], F16, tag="rg", name="rg")
        nc.sync.dma_start(rg[:], dram["cc_out"].rearrange(
            "(x p) f -> p x f", p=128))
        for k in range(N_CORES):
            for cc in range(2):
                blk = rg[:, k * 2 + cc, :]
                nc.vector.tensor_copy(
                    rawAllI[cc][:, :, 8 * k:8 * k + 8],
                    blk[:, 0:72].rearrange("c (g b) -> c g b", b=8))
                nc.vector.tensor_copy(rawAllH[cc][:, 8 * k:8 * k + 8],
                                      blk[:, 72:80])
        gI = []
        for dc in range(2):
            ps = ps_g.tile([128, 64], F32, tag="gI", name=f"gI{dc}")
            n = 0
            for cc in range(2):
                for g in range(NGRP):
                    nc.tensor.matmul(
                        ps[:], gapw[:, cc, g, dc * 128:(dc + 1) * 128],
                        rawAllI[cc][:, g, :],
                        start=(n == 0), stop=(n == 17))
                    n += 1
            gI.append(ps)
        for gi in range(3):
            for dc in range(2):
                t1 = gate.tile([128, 64], F16, tag="t1", bufs=2, name="t1")
                nc.vector.tensor_scalar_mul(t1[:], gI[dc][:],
                                            vecs[:, dc, 3 * gi:3 * gi + 1])
                t2 = gate.tile([128, 64], F16, tag="t2", bufs=2, name="t2")
                nc.vector.tensor_scalar_mul(
                    t2[:], rawAllH[dc][:],
                    vecs[:, dc, 3 * gi + 1:3 * gi + 2])
                nc.vector.tensor_tensor(out=t1[:], in0=t1[:], in1=t2[:],
                                        op=ALU.add)
                gt = gate.tile([128, 64], F16, tag="gt", bufs=2,
                               name=f"gate{gi}{dc}")
                nc.scalar.activation(gt[:], t1[:], AF.Sigmoid,
                                     bias=vecs[:, dc, 3 * gi + 2:3 * gi + 3])
                pt = ps_aux.tile([64, 128], F16, tag="ptr")
                nc.tensor.transpose(pt[:], gt[:], identb[:])
                gtT = gate.tile([64, 128], F16, tag="gtT", bufs=2, name="gtT")
                nc.vector.tensor_copy(gtT[:], pt[:])
                psl = ps_aux.tile([128, BL * H], F32, tag="psl")
                nc.tensor.matmul(psl[:], gtT[:], smat[:], start=True,
                                 stop=True)
                st = gate.tile([128, BL * H], F16, tag=f"sel{gi}{dc}",
                               name=f"sel{gi}{dc}")
                nc.vector.tensor_copy(st[:], psl[:])
                sel[(gi, dc)] = st

    # ---- conv windows ------------------------------------------------------
    g30 = {}     # (j, dc) -> f16 tile [128, 840]

    def emit_conv(j, dc, ms_per_win=0, ew_inline=False):
        gt = g30p.tile([128, 840], F16, tag="g30", name=f"g30_{j}_{dc}")
        g30[(j, dc)] = gt
        for (r0, nr) in WINDOWS:
            nw = nr * 30
            pc = ps_conv.tile([128, 240], F32, tag="pconv", name="pconv")
            s0 = 30 * (1 + r0) + 1
            first = True
            for term, xnm_in, xnm_hd in ((0, "xinh", "xhdh"),
                                         (1, "xinh", "xhdh"),
                                         (2, "xinr", "xhdr")):
                for conv, xnm in ((0, xnm_in), (1, xnm_hd)):
                    xtile = xt[(j, xnm)]
                    for t, (kh, kw) in enumerate(TAPS):
                        st = 30 * (1 + r0 + kh - 1) + 1 + (kw - 1)
                        nc.tensor.matmul(pc[:, 0:nw], wblk(term, conv, t, dc),
                                         xtile[:, :, st:st + nw],
                                         start=first, stop=False,
                                         perf_mode=DR)
                        first = False
            # gateBias last so the first windows don't wait on its load
            nc.tensor.matmul(pc[:, 0:nw], identp[:],
                             gb30[:, dc, :, s0:s0 + nw], start=False,
                             stop=True, perf_mode=DR)
            nc.scalar.activation(gt[:, 30 * r0:30 * r0 + nw], pc[:, 0:nw],
                                 AF.Tanh, scale=1.0 / SW)
            drain_ms(ms_per_win)
            if ew_inline:
                emit_ew_rows(j, dc, r0, nr)

    # ---- elementwise -------------------------------------------------------
    def cm(ap, nw=W):
        # [128, nw*28] column-major -> [128, w, h]
        return ap.rearrange("c (w h) -> c w h", h=H)

    def gsel(gi, dc, j, nw=W):
        return sel[(gi, dc)][:, H * j:H * (j + 1)].rearrange(
            "c (o h) -> c o h", o=1).to_broadcast([128, nw, H])

    def emit_ew(j, dc, chunks=1):
        gv_all = g30[(j, dc)][:].rearrange("c (h w) -> c w h", w=30)
        bounds = [W * c // chunks for c in range(chunks + 1)]
        for ci in range(chunks):
            w0, w1 = bounds[ci], bounds[ci + 1]
            nw = w1 - w0
            cs = slice(w0 * H, w1 * H)
            gv = gv_all[:, w0:w1, :]
            sp3 = cm(spt[j][:, dc, cs], nw)
            gb_ = lambda gi: gsel(gi, dc, j, nw)
            s1 = ewp.tile([128, PIX], F16, tag="s1", name="s1")
            nc.vector.tensor_tensor(out=cm(s1[:, cs], nw), in0=sp3,
                                    in1=gb_(1), op=ALU.mult)
            s2 = ewp.tile([128, PIX], F16, tag="s2", name="s2")
            nc.vector.tensor_tensor(out=cm(s2[:, cs], nw), in0=gv,
                                    in1=gb_(0), op=ALU.mult)
            stt = outp.tile([128, PIX], F16, tag="st", name=f"st{j}_{dc}")
            nc.vector.tensor_tensor(out=stt[:, cs], in0=s1[:, cs],
                                    in1=s2[:, cs], op=ALU.add)
            nc.sync.dma_start(dram["st"][j, dc][:, cs], stt[:, cs])
            th = ewp.tile([128, PIX], F16, tag="th", name="th")
            nc.scalar.activation(th[:, cs], stt[:, cs], AF.Tanh)
            hd = outp.tile([128, PIX], F16, tag="hd", name=f"hd{j}_{dc}")
            nc.vector.tensor_tensor(out=cm(hd[:, cs], nw), in0=cm(th[:, cs], nw),
                                    in1=gb_(2), op=ALU.mult)
            nc.sync.dma_start(dram["hd"][j, dc][:, cs], hd[:, cs])

    def emit_ew_rows(j, dc, r0, nr):
        # row-window elementwise: all APs sliced to h in [r0, r0+nr)
        hs = slice(r0, r0 + nr)
        gv = g30[(j, dc)][:].rearrange("c (h w) -> c w h", w=30)[:, 0:W, hs]

        def gb_(gi):
            return sel[(gi, dc)][:, H * j + r0:H * j + r0 + nr].rearrange(
                "c (o h) -> c o h", o=1).to_broadcast([128, W, nr])

        sp3 = cm(spt[j][:, dc, :])[:, :, hs]
        s1 = ewp.tile([128, PIX], F16, tag="s1", name="s1")
        nc.vector.tensor_tensor(out=cm(s1[:])[:, :, hs], in0=sp3, in1=gb_(1),
                                op=ALU.mult)
        s2 = ewp.tile([128, PIX], F16, tag="s2", name="s2")
        nc.vector.tensor_tensor(out=cm(s2[:])[:, :, hs], in0=gv, in1=gb_(0),
                                op=ALU.mult)
        stt = outp.tile([128, PIX], F16, tag="st", name=f"st{j}_{dc}")
        nc.vector.tensor_tensor(out=cm(stt[:])[:, :, hs],
                                in0=cm(s1[:])[:, :, hs],
                                in1=cm(s2[:])[:, :, hs], op=ALU.add)
        nc.sync.dma_start(
            dram["st"][j, dc].rearrange("p (w h) -> p w h", h=H)[:, :, hs],
            cm(stt[:])[:, :, hs])
        th = ewp.tile([128, PIX], F16, tag="th", name="th")
        nc.scalar.activation(cm(th[:])[:, :, hs], cm(stt[:])[:, :, hs],
                             AF.Tanh)
        hd = outp.tile([128, PIX], F16, tag="hd", name=f"hd{j}_{dc}")
        nc.vector.tensor_tensor(out=cm(hd[:])[:, :, hs],
                                in0=cm(th[:])[:, :, hs], in1=gb_(2),
                                op=ALU.mult)
        nc.sync.dma_start(
            dram["hd"][j, dc].rearrange("p (w h) -> p w h", h=H)[:, :, hs],
            cm(hd[:])[:, :, hs])

    # ======================= emission schedule ==============================
    load_x(0)
    load_x(1)
    for j in range(BL):
        load_nat(j)
    for j in range(BL):
        load_sp(j)

    msq = deque((j, tt, cc) for j in range(BL) for tt in range(2)
                for cc in range(2))

    def drain_ms(n):
        for _ in range(min(n, len(msq))):
            emit_msum(*msq.popleft())

    # convs j0/j1; masked sums drain once their nat tiles are in
    emit_conv(0, 0, ms_per_win=6)
    emit_conv(0, 1, ms_per_win=6)
    drain_ms(len(msq))
    emit_cc_store_and_allgather()
    load_x(2)
    load_x(3)
    emit_conv(1, 0)
    emit_conv(1, 1)

    # j2 dc0 conv gives the collective time to land
    emit_conv(2, 0)
    emit_gates()
    for j in (0, 1):
        for dc in range(2):
            emit_ew(j, dc)
    load_x(4)
    emit_ew(2, 0)
    emit_conv(2, 1)
    emit_ew(2, 1)

    for j in range(3, BL):
        if j + 2 < BL:
            load_x(j + 2)
        last = (j == BL - 1)
        emit_conv(j, 0)
        emit_ew(j, 0)
        emit_conv(j, 1, ew_inline=last)
        if not last:
            emit_ew(j, 1)

    ctx.close()


# ---------------------------------------------------------------------------
_NC_CACHE = None


def _get_nc():
    global _NC_CACHE
    if _NC_CACHE is None:
        nc = build_nc()
        _split_excess_sem_waits(nc)
        _NC_CACHE = nc
    return _NC_CACHE


# ---------------------------------------------------------------------------
def _make_in_maps(inputs):
    import ml_dtypes
    f32 = np.float32
    E4 = ml_dtypes.float8_e4m3
    F16N = np.float16

    x = np.ascontiguousarray(inputs["inputs"], dtype=f32)
    hp = np.ascontiguousarray(inputs["hidden_prev"], dtype=f32)
    sp = np.ascontiguousarray(inputs["state_prev"], dtype=f32)
    wI = np.ascontiguousarray(inputs["wconvInput"], dtype=f32)
    wH = np.ascontiguousarray(inputs["wconvHidden"], dtype=f32)
    gb = np.ascontiguousarray(inputs["gateBias"], dtype=f32)

    def q8(a):
        return np.asarray(a, f32).astype(E4)

    def split(a):
        hi = q8(a)
        res = q8((a - hi.astype(f32)) * SXR)
        return hi, res

    xin_hi, xin_res = split(x)          # [B, H, W, C]
    xhd_hi, xhd_res = split(hp)

    def chan30(a):
        # [B, H, W, C] e4m3 -> [B, 2, 128, G30] channel-major padded 30-grid
        out = np.zeros((B, CIN, 30, 30), dtype=E4)
        out[:, :, 1:29, 1:29] = np.ascontiguousarray(
            a.transpose(0, 3, 1, 2))
        out = out.reshape(B, 2, 128, 900)
        pad = np.zeros((B, 2, 128, G30 - 900), dtype=E4)
        return np.concatenate([out, pad], axis=-1)

    x30 = {"xinh": chan30(xin_hi), "xinr": chan30(xin_res),
           "xhdh": chan30(xhd_hi), "xhdr": chan30(xhd_res)}

    def natlay(a):
        # [B, H, W, C] e4m3 -> [B, 112, 7*256]
        return np.ascontiguousarray(
            a.reshape(B, 7, 112, CIN).transpose(0, 2, 1, 3)).reshape(
                B, 112, 7 * 256)

    nat = np.stack([natlay(xin_hi), natlay(xhd_hi)], axis=1)  # [B,2,112,1792]

    sp_cm = np.ascontiguousarray(sp.transpose(0, 3, 2, 1)).reshape(
        B, 2, 128, PIX).astype(F16N)

    # weight packs
    def packs(w):
        W256 = SW * w
        Ws = q8(W256)
        QR = q8(W256 - Ws.astype(f32))
        WB = q8(W256 / SXR)
        return Ws, QR, WB

    pI = packs(wI)
    pH = packs(wH)
    wpk = np.empty((128, 3 * 2 * 9 * 2 * 256), dtype=E4)
    for term in range(3):
        for conv, p in ((0, pI), (1, pH)):
            arr = p[term]   # [3, 3, 256, 256]
            for t, (kh, kw) in enumerate(TAPS):
                for dc in range(2):
                    base = (((dc * 3 + term) * 2 + conv) * 9 + t) * 256
                    blk = arr[kh, kw][:, dc * 128:(dc + 1) * 128]  # [256,128]
                    wpk[:, base:base + 256] = np.ascontiguousarray(
                        blk.reshape(2, 128, 128).transpose(1, 0, 2)).reshape(
                            128, 256)

    gbs = np.zeros((CIN, 30, 30), dtype=f32)
    gbs[:, 1:29, 1:29] = (SW * gb).transpose(2, 0, 1)
    gb_hi = q8(gbs)
    gb_lo = q8(gbs - gb_hi.astype(f32))
    gb30 = np.zeros((2, 2, 128, G30), dtype=E4)
    gb30[:, 0, :, :900] = gb_hi.reshape(2, 128, 900)
    gb30[:, 1, :, :900] = gb_lo.reshape(2, 128, 900)

    gapw = np.empty((128, 2 * NGRP * 256), dtype=F16N)
    for g, taps in enumerate(GAP_TAPSETS):
        comb = np.zeros((CIN, D), f32)
        for t in taps:
            comb += wI[t // 3, t % 3]
        comb /= float(PIX)
        for cc in range(2):
            gapw[:, (cc * NGRP + g) * 256:(cc * NGRP + g) * 256 + 256] = \
                comb[cc * 128:(cc + 1) * 128, :].astype(F16N)

    m = np.zeros((PIX, MCOL), f32)
    hw = np.arange(PIX)
    r, c = hw // W, hw % W
    m[:, 0] = 1.0
    m[r == 0, 1] = -1.0
    m[r == H - 1, 2] = -1.0
    m[c == 0, 3] = -1.0
    m[c == W - 1, 4] = -1.0
    m[(r == 0) & (c == 0), 5] = 1.0
    m[(r == 0) & (c == W - 1), 6] = 1.0
    m[(r == H - 1) & (c == 0), 7] = 1.0
    m[(r == H - 1) & (c == W - 1), 8] = 1.0
    m[:, 9] = 1.0
    masks = np.ascontiguousarray(
        m.reshape(7, 112, MCOL).transpose(1, 0, 2)).reshape(
            112, 7 * MCOL).astype(E4)

    vecs = np.zeros((128, 2, 9), f32)
    for gi, (wx, wh, bi) in enumerate((("wxi", "whi", "inputBias"),
                                       ("wxf", "whf", "forgetBias"),
                                       ("wxo", "who", "outputBias"))):
        for dc in range(2):
            s = slice(dc * 128, (dc + 1) * 128)
            vecs[:, dc, 3 * gi] = np.asarray(inputs[wx], f32)[s]
            vecs[:, dc, 3 * gi + 1] = np.asarray(inputs[wh], f32)[s] / PIX
            vecs[:, dc, 3 * gi + 2] = np.asarray(inputs[bi], f32)[s]

    shared = {"wpk": wpk, "gb30": gb30, "gapw": gapw, "masks": masks,
              "vecs": vecs}

    in_maps = []
    for k in range(N_CORES):
        sl = slice(k * BL, (k + 1) * BL)
        mm = dict(shared)
        for nm in ("xinh", "xinr", "xhdh", "xhdr"):
            mm[nm] = x30[nm][sl]
        mm["sp"] = sp_cm[sl]
        mm["nat"] = nat[sl]
        smat = np.zeros((64, BL * H), dtype=F16N)
        for j in range(BL):
            for h in range(H):
                smat[(H * (BL * k + j) + h) % B, H * j + h] = 1.0
        mm["smat"] = smat
        in_maps.append(mm)
    return in_maps


def kernel(**inputs):
    nc = _get_nc()
    in_maps = _make_in_maps(inputs)
    res = run_bass_kernel_spmd(nc, in_maps, core_ids=list(range(N_CORES)))

    def unshard(name):
        full = np.concatenate([res.results[k][name] for k in range(N_CORES)],
                              axis=0)
        # [B, 2, 128, 784] f16 (d-major, col-major pixels) -> [B, H, W, D]
        arr = full.astype(np.float32).reshape(B, 2, 128, W, H)
        return np.ascontiguousarray(arr.transpose(0, 4, 3, 1, 2)).reshape(
            B, H, W, D)

    return unshard("hd_out"), unshard("st_out")


# revision 27
# speedup vs baseline: 1.0431x; 1.0431x over previous
"""Trainium2 Bass kernel for nn_DeformableConvLSTMCell_33895881900284.

Full (unsharded) inputs in, full outputs out. Data-parallel over batch across
8 NeuronCores (8 batches per core), conv weights / gate params replicated.

Math per the reference:
  outI  = conv3x3_same(inputs, wconvInput)
  g     = tanh(outI + conv3x3_same(hidden_prev, wconvHidden) + gateBias)
  gapI  = mean_hw(outI);  gapH = mean_hw(hidden_prev)          # [B, D]
  i/f/o = sigmoid(wx*gapI + wh*gapH + bias)                    # [B, D]
  tiled gate: value used at (b, h, w, c) is gate[(28*b + h) % 64, c]
  state  = f*state_prev + i*g;  hidden = o*tanh(state)

Design (PE-bound; conv is ~95% of PE work):
  - conv runs in fp8(e4m3) DoubleRow mode: each matmul contracts K=256 (both
    128-channel chunks) at 0.5 cycles/output-column.  Accuracy is recovered
    with a 3-term residual scheme, all power-of-2 scaled so one PSUM
    accumulation holds everything:
        psum = x_hi*(W* + QR) + XR*WB + 256*gateBias
    where W* = q8(256 w), QR = q8(256 w - W*), x_hi = q8(x),
    XR = q8(32 (x - x_hi)), WB = q8(8 w); then g = tanh(psum / 256).
    (measured rel-err ~1.2e-2 vs the 2e-2 gate; plain fp8 is 0.12)
  - activations live channel-major on a padded 30x30 "30-grid" so every tap's
    moving operand is one contiguous 1-D slice (3-D AP as DoubleRow needs);
    output windows are nr rows x 30 cols with 2 junk columns per row.
  - gateBias is injected into PSUM by a bf16/f16 identity-stationary matmul
    (moving = channel-major gateBias slice), so no vector-engine bias add;
    ACT applies tanh with scale=1/256 straight from PSUM.
  - GAP path: masked pixel sums of the raw fp8 inputs ride tiny matmuls
    (stationary = natural-layout tile, moving = signed 0/+-1 masks, 16 output
    cols), are AllGather'ed (raw, unnormalized), and every core rebuilds the
    gates for all 64 batches locally: 18 small matmuls (gapI tap-combos by
    linearity of conv+GAP), tensor_scalar gate algebra, sigmoid, a PE
    transpose, and a per-core selection-matrix matmul that realizes the
    (28 b + h) % 64 gate scrambling with plain data (SPMD-safe).
  - elementwise runs column-major (pixel = 28*w + h) in f16 so the per-row
    gate broadcast is a middle-axis stride-0 AP and DVE gets its 2x mode for
    the add; outputs are stored f16 column-major and the host restores NHWC.
"""
import numpy as np

import bass_rust
import concourse.bass as bass
import concourse.mybir as mybir
import concourse.tile as tile
from concourse.bass_utils import run_bass_kernel_spmd

F32 = mybir.dt.float32
F16 = mybir.dt.float16
FP8 = mybir.dt.float8e4
AF = mybir.ActivationFunctionType
ALU = mybir.AluOpType
DR = mybir.MatmulPerfMode.DoubleRow

N_CORES = 8
B, H, W, CIN, D = 64, 28, 28, 256, 256
BL = B // N_CORES
PIX = H * W                    # 784
G30 = 912                      # padded 30-grid row-major length (30*30 -> 912)
SW = 256.0                     # weight scale
SXR = 32.0                     # activation-residual scale
WINDOWS = [(0, 8), (8, 8), (16, 8), (24, 4)]   # (r0, nr)
TAPS = [(kh, kw) for kh in range(3) for kw in range(3)]
NGRP = 9                       # gapI mask groups
MCOL = 16                      # mask columns (9 gapI + 1 full-sum + pad)

# gapI tap combos per mask group (group signs live in the masks)
GAP_TAPSETS = [list(range(9)), [6, 7, 8], [0, 1, 2], [2, 5, 8], [0, 3, 6],
               [8], [6], [2], [0]]

# ---------------------------------------------------------------------------
# walrus fixup: split semaphore waits that exceed the per-instruction budget
MAX_WAITS = 1


def _split_excess_sem_waits(nc):
    counter = [0]
    for fn in nc.m.functions:
        for bb in fn.blocks:
            insts = bb.instructions
            i = 0
            while i < len(insts):
                inst = insts[i]
                si = inst.sync_info
                if si is not None and si.on_wait and len(si.on_wait) > MAX_WAITS:
                    waits = list(si.on_wait)
                    excess = waits[:-MAX_WAITS]
                    keep = waits[-MAX_WAITS:]
                    new_insts = []
                    for j in range(0, len(excess), MAX_WAITS):
                        chunk = excess[j:j + MAX_WAITS]
                        noop = mybir.InstNoOp(
                            name=f"I-waitsplit-{counter[0]}", ins=[], outs=[])
                        counter[0] += 1
                        noop.engine = inst.engine
                        noop.sync_info = bass_rust.SyncInfo(
                            on_wait=chunk, on_update=[])
                        nc.register_instruction(noop)
                        new_insts.append(noop)
                    inst.sync_info = bass_rust.SyncInfo(
                        on_wait=keep, on_update=list(si.on_update))
                    insts[i:i] = new_insts
                    i += len(new_insts)
                i += 1
    return nc


# ---------------------------------------------------------------------------
def build_nc():
    nc = bass.Bass("TRN2", target_bir_lowering=False, debug=False,
                   num_devices=N_CORES)

    dram = {}
    # channel-major padded 30-grid fp8 activations (hi + scaled residual)
    for nm in ("xinh", "xinr", "xhdh", "xhdr"):
        dram[nm] = nc.dram_tensor(nm, [BL, 2, 128, G30], FP8,
                                  kind="ExternalInput").ap()
    # state_prev, column-major f16
    dram["sp"] = nc.dram_tensor("sp", [BL, 2, 128, PIX], F16,
                                kind="ExternalInput").ap()
    # natural-layout fp8 copies for the GAP masked sums (0=inputs, 1=hidden)
    dram["nat"] = nc.dram_tensor("nat", [BL, 2, 112, 7 * 256], FP8,
                                 kind="ExternalInput").ap()
    # fp8 conv weight packs: (term, conv, tap, dc) blocks of [128, 2, 128]
    dram["wpk"] = nc.dram_tensor("wpk", [128, 3 * 2 * 9 * 2 * 256], FP8,
                                 kind="ExternalInput").ap()
    # channel-major 30-grid gateBias, pre-scaled by SW, fp8 hi+lo planes
    dram["gb30"] = nc.dram_tensor("gb30", [2, 2, 128, G30], FP8,
                                  kind="ExternalInput").ap()
    # gapI tap-combo weights (already / 784), f16: (cc, g) blocks [128, 256]
    dram["gapw"] = nc.dram_tensor("gapw", [128, 2 * NGRP * 256], F16,
                                  kind="ExternalInput").ap()
    # signed GAP masks
    dram["masks"] = nc.dram_tensor("masks", [112, 7 * MCOL], FP8,
                                   kind="ExternalInput").ap()
    # per-core gate selection matrix
    dram["smat"] = nc.dram_tensor("smat", [64, BL * H], F16,
                                  kind="ExternalInput").ap()
    # gate vectors: per dc 9 cols (wxi, whi/784, ib, wxf, whf/784, fb, ...)
    dram["vecs"] = nc.dram_tensor("vecs", [128, 2, 9], F32,
                                  kind="ExternalInput").ap()
    # outputs, column-major f16
    dram["st"] = nc.dram_tensor("st_out", [BL, 2, 128, PIX], F16,
                                kind="ExternalOutput").ap()
    dram["hd"] = nc.dram_tensor("hd_out", [BL, 2, 128, PIX], F16,
                                kind="ExternalOutput").ap()
    # collective buffers (raw GAP sums)
    dram["cc_in"] = nc.dram_tensor("cc_in", [256, 80], F16,
                                   kind="Internal").ap()
    dram["cc_out"] = nc.dram_tensor("cc_out", [N_CORES * 256, 80], F16,
                                    kind="Internal", addr_space="Shared").ap()

    ctx_mgr = nc.allow_low_precision("fp8 DoubleRow conv + f16 elementwise")
    ctx_mgr.__enter__()
    with tile.TileContext(nc) as tc:
        _build_body(nc, tc, dram)
    ctx_mgr.__exit__(None, None, None)
    return nc


def _build_body(nc, tc, dram):
    from collections import deque
    from contextlib import ExitStack
    ctx = ExitStack()
    pool = lambda **kw: ctx.enter_context(tc.tile_pool(**kw))

    const = pool(name="const", bufs=1)
    wts = pool(name="wts", bufs=1)
    xp = pool(name="xp", bufs=4)         # fp8 activation tiles
    natp = pool(name="natp", bufs=8)     # natural fp8 tiles for GAP
    spp = pool(name="spp", bufs=8)       # state_prev tiles (all resident)
    g30p = pool(name="g30p", bufs=8)     # tanh outputs per (j, dc)
    ewp = pool(name="ewp", bufs=3)       # elementwise temps
    outp = pool(name="outp", bufs=3)     # st/hd staging
    gate = pool(name="gate", bufs=1)     # persistent gate pipeline tiles
    ps_conv = pool(name="ps_conv", bufs=2, space="PSUM")
    ps_ms = pool(name="ps_ms", bufs=2, space="PSUM")
    ps_g = pool(name="ps_g", bufs=2, space="PSUM")
    ps_aux = pool(name="ps_aux", bufs=1, space="PSUM")

    # ---- constants ---------------------------------------------------------
    # identity built on-chip (Pool), copied to f16
    ident = const.tile([128, 128], F32, tag="ident")
    nc.gpsimd.memset(ident[:], 1.0)
    nc.gpsimd.affine_select(ident[:], ident[:], [[-1, 128]],
                            ALU.is_equal, 0.0, base=0, channel_multiplier=1)
    identb = const.tile([128, 128], F16, tag="identb")
    nc.vector.tensor_copy(identb[:], ident[:])

    masks = const.tile([112, 7, MCOL], FP8, tag="masks")
    nc.gpsimd.dma_start(masks[:], dram["masks"].rearrange(
        "p (g m) -> p g m", m=MCOL))

    # conv weights: dc-major packing; dc0 chunks on SP, dc1 on Pool so the
    # Activation engine never runs bulk DMA (its queue must stay clear to
    # drain conv PSUMs promptly)
    wpk = wts.tile([128, 3 * 2 * 9 * 2 * 256], FP8, tag="wpk")
    TCH = 9 * 256

    def load_wpk(r):
        for t12 in r:
            nc.sync.dma_start(wpk[:, t12 * TCH:(t12 + 1) * TCH],
                              dram["wpk"][:, t12 * TCH:(t12 + 1) * TCH])

    load_wpk(range(6))
    # pre-warm the ACT table (tanh/sigmoid share a set); eats the 1.4us
    # table-load before the first conv window needs the engine
    actwarm = const.tile([128, 1], F16, tag="actwarm")
    nc.scalar.activation(actwarm[:], identb[:, 0:1], AF.Tanh)

    gb30 = const.tile([128, 2, 2, G30], FP8, tag="gb30")
    nc.sync.dma_start(gb30[:], dram["gb30"].rearrange("d l p f -> p d l f"))
    nc.sync.dma_start(wpk[:, 6 * TCH:9 * TCH], dram["wpk"][:, 6 * TCH:9 * TCH])
    nc.sync.dma_start(wpk[:, 9 * TCH:12 * TCH],
                      dram["wpk"][:, 9 * TCH:12 * TCH])
    identp = const.tile([128, 2, 128], FP8, tag="identp")
    for _l in range(2):
        nc.vector.tensor_copy(identp[:, _l, :], ident[:])

    def wblk(term, conv, t, dc):
        base = (((dc * 3 + term) * 2 + conv) * 9 + t) * 256
        return wpk[:, base:base + 256].rearrange("c (i d) -> c i d", i=2)

    gapw = wts.tile([128, 2, NGRP, 256], F16, tag="gapw")
    nc.sync.dma_start(gapw[:], dram["gapw"].rearrange(
        "c (i g d) -> c i g d", i=2, g=NGRP))
    smat = const.tile([64, BL * H], F16, tag="smat")
    nc.sync.dma_start(smat[:], dram["smat"])
    vecs = const.tile([128, 2, 9], F32, tag="vecs")
    nc.sync.dma_start(vecs[:], dram["vecs"])

    # ---- per-batch loads ---------------------------------------------------
    xt = {}      # (j, nm) -> fp8 tile [128, 2, G30]
    spt = {}     # j -> f16 tile [128, 2, PIX]
    natt = {}    # j -> fp8 tile [112, 2, 7, 256]

    def load_x(j):
        for nm in ("xinh", "xinr", "xhdh", "xhdr"):
            t = xp.tile([128, 2, G30], FP8, tag=nm, name=f"{nm}{j}")
            nc.gpsimd.dma_start(t[:], dram[nm][j].rearrange("i p f -> p i f"))
            xt[(j, nm)] = t

    def load_nat(j):
        t = natp.tile([112, 2, 7, 256], FP8, tag="nat", name=f"nat{j}")
        nc.gpsimd.dma_start(t[:], dram["nat"][j].rearrange(
            "i p (g c) -> p i g c", c=256))
        natt[j] = t

    def load_sp(j):
        t = spp.tile([128, 2, PIX], F16, tag="sp", name=f"sp{j}")
        nc.gpsimd.dma_start(t[:], dram["sp"][j].rearrange("i p f -> p i f"))
        spt[j] = t

    # ---- GAP masked sums ---------------------------------------------------
    raw = [gate.tile([128, 80], F16, tag=f"raw{cc}", name=f"raw{cc}")
           for cc in range(2)]

    def emit_msum(j, tt, cc):
        ps = ps_ms.tile([128, MCOL], F32, tag="ms")
        for g in range(0, 6, 2):
            nc.tensor.matmul(
                ps[:], natt[j][:, tt, g:g + 2, cc * 128:(cc + 1) * 128],
                masks[:, g:g + 2, :], start=(g == 0), stop=False,
                perf_mode=DR)
        nc.tensor.matmul(ps[:], natt[j][:, tt, 6, cc * 128:(cc + 1) * 128],
                         masks[:, 6, :], start=False, stop=True)
        if tt == 0:
            dst = raw[cc][:, 0:72].rearrange("c (g b) -> c g b", b=BL)
            nc.vector.tensor_copy(
                dst[:, :, j:j + 1],
                ps[:, 0:NGRP].rearrange("c (n o) -> c n o", o=1))
        else:
            nc.vector.tensor_copy(raw[cc][:, 72 + j:73 + j],
                                  ps[:, NGRP:NGRP + 1])

    # ---- collective --------------------------------------------------------
    def emit_cc_store_and_allgather():
        for cc in range(2):
            nc.sync.dma_start(dram["cc_in"][cc * 128:(cc + 1) * 128, :],
                              raw[cc][:])
        nc.gpsimd.collective_compute(
            "AllGather", ALU.bypass, replica_groups=[list(range(N_CORES))],
            ins=[dram["cc_in"][:]], outs=[dram["cc_out"][:]])

    # ---- gates for all 64 batches ------------------------------------------
    sel = {}     # (gate_idx, dc) -> [128, BL*H] f16

    def emit_gates():
        rawAllI = [gate.tile([128, NGRP, 64], F16, tag=f"raI{cc}",
                             name=f"raI{cc}") for cc in range(2)]
        rawAllH = [gate.tile([128, 64], F16, tag=f"raH{cc}", name=f"raH{cc}")
                   for cc in range(2)]
        rg = gate.tile([128, 16, 80], F16, tag="rg", name="rg")
        nc.sync.dma_start(rg[:], dram["cc_out"].rearrange(
            "(x p) f -> p x f", p=128))
        for k in range(N_CORES):
            for cc in range(2):
                blk = rg[:, k * 2 + cc, :]
                nc.vector.tensor_copy(
                    rawAllI[cc][:, :, 8 * k:8 * k + 8],
                    blk[:, 0:72].rearrange("c (g b) -> c g b", b=8))
                nc.vector.tensor_copy(rawAllH[cc][:, 8 * k:8 * k + 8],
                                      blk[:, 72:80])
        gI = []
        for dc in range(2):
            ps = ps_g.tile([128, 64], F32, tag="gI", name=f"gI{dc}")
            n = 0
            for cc in range(2):
                for g in range(NGRP):
                    nc.tensor.matmul(
                        ps[:], gapw[:, cc, g, dc * 128:(dc + 1) * 128],
                        rawAllI[cc][:, g, :],
                        start=(n == 0), stop=(n == 17))
                    n += 1
            gI.append(ps)
        for gi in range(3):
            for dc in range(2):
                t1 = gate.tile([128, 64], F16, tag="t1", bufs=2, name="t1")
                nc.vector.tensor_scalar_mul(t1[:], gI[dc][:],
                                            vecs[:, dc, 3 * gi:3 * gi + 1])
                t2 = gate.tile([128, 64], F16, tag="t2", bufs=2, name="t2")
                nc.vector.tensor_scalar_mul(
                    t2[:], rawAllH[dc][:],
                    vecs[:, dc, 3 * gi + 1:3 * gi + 2])
                nc.vector.tensor_tensor(out=t1[:], in0=t1[:], in1=t2[:],
                                        op=ALU.add)
                gt = gate.tile([128, 64], F16, tag="gt", bufs=2,
                               name=f"gate{gi}{dc}")
                nc.scalar.activation(gt[:], t1[:], AF.Sigmoid,
                                     bias=vecs[:, dc, 3 * gi + 2:3 * gi + 3])
                pt = ps_aux.tile([64, 128], F16, tag="ptr")
                nc.tensor.transpose(pt[:], gt[:], identb[:])
                gtT = gate.tile([64, 128], F16, tag="gtT", bufs=2, name="gtT")
                nc.vector.tensor_copy(gtT[:], pt[:])
                psl = ps_aux.tile([128, BL * H], F32, tag="psl")
                nc.tensor.matmul(psl[:], gtT[:], smat[:], start=True,
                                 stop=True)
                st = gate.tile([128, BL * H], F16, tag=f"sel{gi}{dc}",
                               name=f"sel{gi}{dc}")
                nc.vector.tensor_copy(st[:], psl[:])
                sel[(gi, dc)] = st

    # ---- conv windows ------------------------------------------------------
    g30 = {}     # (j, dc) -> f16 tile [128, 840]

    def emit_conv(j, dc, ms_per_win=0, ew_inline=False):
        gt = g30p.tile([128, 840], F16, tag="g30", name=f"g30_{j}_{dc}")
        g30[(j, dc)] = gt
        for (r0, nr) in WINDOWS:
            nw = nr * 30
            pc = ps_conv.tile([128, 240], F32, tag="pconv", name="pconv")
            s0 = 30 * (1 + r0) + 1
            first = True
            for term, xnm_in, xnm_hd in ((0, "xinh", "xhdh"),
                                         (1, "xinh", "xhdh"),
                                         (2, "xinr", "xhdr")):
                for conv, xnm in ((0, xnm_in), (1, xnm_hd)):
                    xtile = xt[(j, xnm)]
                    for t, (kh, kw) in enumerate(TAPS):
                        st = 30 * (1 + r0 + kh - 1) + 1 + (kw - 1)
                        nc.tensor.matmul(pc[:, 0:nw], wblk(term, conv, t, dc),
                                         xtile[:, :, st:st + nw],
                                         start=first, stop=False,
                                         perf_mode=DR)
                        first = False
            # gateBias last so the first windows don't wait on its load
            nc.tensor.matmul(pc[:, 0:nw], identp[:],
                             gb30[:, dc, :, s0:s0 + nw], start=False,
                             stop=True, perf_mode=DR)
            nc.scalar.activation(gt[:, 30 * r0:30 * r0 + nw], pc[:, 0:nw],
                                 AF.Tanh, scale=1.0 / SW)
            drain_ms(ms_per_win)
            if ew_inline:
                emit_ew_rows(j, dc, r0, nr)

    # ---- elementwise -------------------------------------------------------
    def cm(ap, nw=W):
        # [128, nw*28] column-major -> [128, w, h]
        return ap.rearrange("c (w h) -> c w h", h=H)

    def gsel(gi, dc, j, nw=W):
        return sel[(gi, dc)][:, H * j:H * (j + 1)].rearrange(
            "c (o h) -> c o h", o=1).to_broadcast([128, nw, H])

    def emit_ew(j, dc, chunks=1):
        gv_all = g30[(j, dc)][:].rearrange("c (h w) -> c w h", w=30)
        bounds = [W * c // chunks for c in range(chunks + 1)]
        for ci in range(chunks):
            w0, w1 = bounds[ci], bounds[ci + 1]
            nw = w1 - w0
            cs = slice(w0 * H, w1 * H)
            gv = gv_all[:, w0:w1, :]
            sp3 = cm(spt[j][:, dc, cs], nw)
            gb_ = lambda gi: gsel(gi, dc, j, nw)
            s1 = ewp.tile([128, PIX], F16, tag="s1", name="s1")
            nc.vector.tensor_tensor(out=cm(s1[:, cs], nw), in0=sp3,
                                    in1=gb_(1), op=ALU.mult)
            s2 = ewp.tile([128, PIX], F16, tag="s2", name="s2")
            nc.vector.tensor_tensor(out=cm(s2[:, cs], nw), in0=gv,
                                    in1=gb_(0), op=ALU.mult)
            stt = outp.tile([128, PIX], F16, tag="st", name=f"st{j}_{dc}")
            nc.vector.tensor_tensor(out=stt[:, cs], in0=s1[:, cs],
                                    in1=s2[:, cs], op=ALU.add)
            nc.sync.dma_start(dram["st"][j, dc][:, cs], stt[:, cs])
            th = ewp.tile([128, PIX], F16, tag="th", name="th")
            nc.scalar.activation(th[:, cs], stt[:, cs], AF.Tanh)
            hd = outp.tile([128, PIX], F16, tag="hd", name=f"hd{j}_{dc}")
            nc.vector.tensor_tensor(out=cm(hd[:, cs], nw), in0=cm(th[:, cs], nw),
                                    in1=gb_(2), op=ALU.mult)
            nc.sync.dma_start(dram["hd"][j, dc][:, cs], hd[:, cs])

    def emit_ew_rows(j, dc, r0, nr):
        # row-window elementwise: all APs sliced to h in [r0, r0+nr)
        hs = slice(r0, r0 + nr)
        gv = g30[(j, dc)][:].rearrange("c (h w) -> c w h", w=30)[:, 0:W, hs]

        def gb_(gi):
            return sel[(gi, dc)][:, H * j + r0:H * j + r0 + nr].rearrange(
                "c (o h) -> c o h", o=1).to_broadcast([128, W, nr])

        sp3 = cm(spt[j][:, dc, :])[:, :, hs]
        s1 = ewp.tile([128, PIX], F16, tag="s1", name="s1")
        nc.vector.tensor_tensor(out=cm(s1[:])[:, :, hs], in0=sp3, in1=gb_(1),
                                op=ALU.mult)
        s2 = ewp.tile([128, PIX], F16, tag="s2", name="s2")
        nc.vector.tensor_tensor(out=cm(s2[:])[:, :, hs], in0=gv, in1=gb_(0),
                                op=ALU.mult)
        stt = outp.tile([128, PIX], F16, tag="st", name=f"st{j}_{dc}")
        nc.vector.tensor_tensor(out=cm(stt[:])[:, :, hs],
                                in0=cm(s1[:])[:, :, hs],
                                in1=cm(s2[:])[:, :, hs], op=ALU.add)
        nc.sync.dma_start(
            dram["st"][j, dc].rearrange("p (w h) -> p w h", h=H)[:, :, hs],
            cm(stt[:])[:, :, hs])
        th = ewp.tile([128, PIX], F16, tag="th", name="th")
        nc.scalar.activation(cm(th[:])[:, :, hs], cm(stt[:])[:, :, hs],
                             AF.Tanh)
        hd = outp.tile([128, PIX], F16, tag="hd", name=f"hd{j}_{dc}")
        nc.vector.tensor_tensor(out=cm(hd[:])[:, :, hs],
                                in0=cm(th[:])[:, :, hs], in1=gb_(2),
                                op=ALU.mult)
        nc.sync.dma_start(
            dram["hd"][j, dc].rearrange("p (w h) -> p w h", h=H)[:, :, hs],
            cm(hd[:])[:, :, hs])

    # ======================= emission schedule ==============================
    load_x(0)
    load_x(1)
    for j in range(BL):
        load_nat(j)

    msq = deque((j, tt, cc) for j in range(BL) for tt in range(2)
                for cc in range(2))

    def drain_ms(n):
        for _ in range(min(n, len(msq))):
            emit_msum(*msq.popleft())

    # convs j0/j1; masked sums drain once their nat tiles are in
    emit_conv(0, 0, ms_per_win=6)
    emit_conv(0, 1, ms_per_win=6)
    load_x(2)
    load_x(3)
    drain_ms(len(msq))
    emit_cc_store_and_allgather()
    for j in range(BL):
        load_sp(j)
    emit_conv(1, 0)
    emit_conv(1, 1)

    # j2 dc0 conv gives the collective time to land
    emit_conv(2, 0)
    emit_gates()
    for j in (0, 1):
        for dc in range(2):
            emit_ew(j, dc)
    load_x(4)
    emit_ew(2, 0)
    emit_conv(2, 1)
    emit_ew(2, 1)

    for j in range(3, BL):
        if j + 2 < BL:
            load_x(j + 2)
        last = (j == BL - 1)
        emit_conv(j, 0)
        emit_ew(j, 0)
        emit_conv(j, 1, ew_inline=last)
        if not last:
            emit_ew(j, 1)

    ctx.close()


# ---------------------------------------------------------------------------
_NC_CACHE = None


def _get_nc():
    global _NC_CACHE
    if _NC_CACHE is None:
        nc = build_nc()
        _split_excess_sem_waits(nc)
        _NC_CACHE = nc
    return _NC_CACHE


# ---------------------------------------------------------------------------
def _make_in_maps(inputs):
    import ml_dtypes
    f32 = np.float32
    E4 = ml_dtypes.float8_e4m3
    F16N = np.float16

    x = np.ascontiguousarray(inputs["inputs"], dtype=f32)
    hp = np.ascontiguousarray(inputs["hidden_prev"], dtype=f32)
    sp = np.ascontiguousarray(inputs["state_prev"], dtype=f32)
    wI = np.ascontiguousarray(inputs["wconvInput"], dtype=f32)
    wH = np.ascontiguousarray(inputs["wconvHidden"], dtype=f32)
    gb = np.ascontiguousarray(inputs["gateBias"], dtype=f32)

    def q8(a):
        return np.asarray(a, f32).astype(E4)

    def split(a):
        hi = q8(a)
        res = q8((a - hi.astype(f32)) * SXR)
        return hi, res

    xin_hi, xin_res = split(x)          # [B, H, W, C]
    xhd_hi, xhd_res = split(hp)

    def chan30(a):
        # [B, H, W, C] e4m3 -> [B, 2, 128, G30] channel-major padded 30-grid
        out = np.zeros((B, CIN, 30, 30), dtype=E4)
        out[:, :, 1:29, 1:29] = np.ascontiguousarray(
            a.transpose(0, 3, 1, 2))
        out = out.reshape(B, 2, 128, 900)
        pad = np.zeros((B, 2, 128, G30 - 900), dtype=E4)
        return np.concatenate([out, pad], axis=-1)

    x30 = {"xinh": chan30(xin_hi), "xinr": chan30(xin_res),
           "xhdh": chan30(xhd_hi), "xhdr": chan30(xhd_res)}

    def natlay(a):
        # [B, H, W, C] e4m3 -> [B, 112, 7*256]
        return np.ascontiguousarray(
            a.reshape(B, 7, 112, CIN).transpose(0, 2, 1, 3)).reshape(
                B, 112, 7 * 256)

    nat = np.stack([natlay(xin_hi), natlay(xhd_hi)], axis=1)  # [B,2,112,1792]

    sp_cm = np.ascontiguousarray(sp.transpose(0, 3, 2, 1)).reshape(
        B, 2, 128, PIX).astype(F16N)

    # weight packs
    def packs(w):
        W256 = SW * w
        Ws = q8(W256)
        QR = q8(W256 - Ws.astype(f32))
        WB = q8(W256 / SXR)
        return Ws, QR, WB

    pI = packs(wI)
    pH = packs(wH)
    wpk = np.empty((128, 3 * 2 * 9 * 2 * 256), dtype=E4)
    for term in range(3):
        for conv, p in ((0, pI), (1, pH)):
            arr = p[term]   # [3, 3, 256, 256]
            for t, (kh, kw) in enumerate(TAPS):
                for dc in range(2):
                    base = (((dc * 3 + term) * 2 + conv) * 9 + t) * 256
                    blk = arr[kh, kw][:, dc * 128:(dc + 1) * 128]  # [256,128]
                    wpk[:, base:base + 256] = np.ascontiguousarray(
                        blk.reshape(2, 128, 128).transpose(1, 0, 2)).reshape(
                            128, 256)

    gbs = np.zeros((CIN, 30, 30), dtype=f32)
    gbs[:, 1:29, 1:29] = (SW * gb).transpose(2, 0, 1)
    gb_hi = q8(gbs)
    gb_lo = q8(gbs - gb_hi.astype(f32))
    gb30 = np.zeros((2, 2, 128, G30), dtype=E4)
    gb30[:, 0, :, :900] = gb_hi.reshape(2, 128, 900)
    gb30[:, 1, :, :900] = gb_lo.reshape(2, 128, 900)

    gapw = np.empty((128, 2 * NGRP * 256), dtype=F16N)
    for g, taps in enumerate(GAP_TAPSETS):
        comb = np.zeros((CIN, D), f32)
        for t in taps:
            comb += wI[t // 3, t % 3]
        comb /= float(PIX)
        for cc in range(2):
            gapw[:, (cc * NGRP + g) * 256:(cc * NGRP + g) * 256 + 256] = \
                comb[cc * 128:(cc + 1) * 128, :].astype(F16N)

    m = np.zeros((PIX, MCOL), f32)
    hw = np.arange(PIX)
    r, c = hw // W, hw % W
    m[:, 0] = 1.0
    m[r == 0, 1] = -1.0
    m[r == H - 1, 2] = -1.0
    m[c == 0, 3] = -1.0
    m[c == W - 1, 4] = -1.0
    m[(r == 0) & (c == 0), 5] = 1.0
    m[(r == 0) & (c == W - 1), 6] = 1.0
    m[(r == H - 1) & (c == 0), 7] = 1.0
    m[(r == H - 1) & (c == W - 1), 8] = 1.0
    m[:, 9] = 1.0
    masks = np.ascontiguousarray(
        m.reshape(7, 112, MCOL).transpose(1, 0, 2)).reshape(
            112, 7 * MCOL).astype(E4)

    vecs = np.zeros((128, 2, 9), f32)
    for gi, (wx, wh, bi) in enumerate((("wxi", "whi", "inputBias"),
                                       ("wxf", "whf", "forgetBias"),
                                       ("wxo", "who", "outputBias"))):
        for dc in range(2):
            s = slice(dc * 128, (dc + 1) * 128)
            vecs[:, dc, 3 * gi] = np.asarray(inputs[wx], f32)[s]
            vecs[:, dc, 3 * gi + 1] = np.asarray(inputs[wh], f32)[s] / PIX
            vecs[:, dc, 3 * gi + 2] = np.asarray(inputs[bi], f32)[s]

    shared = {"wpk": wpk, "gb30": gb30, "gapw": gapw, "masks": masks,
              "vecs": vecs}

    in_maps = []
    for k in range(N_CORES):
        sl = slice(k * BL, (k + 1) * BL)
        mm = dict(shared)
        for nm in ("xinh", "xinr", "xhdh", "xhdr"):
            mm[nm] = x30[nm][sl]
        mm["sp"] = sp_cm[sl]
        mm["nat"] = nat[sl]
        smat = np.zeros((64, BL * H), dtype=F16N)
        for j in range(BL):
            for h in range(H):
                smat[(H * (BL * k + j) + h) % B, H * j + h] = 1.0
        mm["smat"] = smat
        in_maps.append(mm)
    return in_maps


def kernel(**inputs):
    nc = _get_nc()
    in_maps = _make_in_maps(inputs)
    res = run_bass_kernel_spmd(nc, in_maps, core_ids=list(range(N_CORES)))

    def unshard(name):
        full = np.concatenate([res.results[k][name] for k in range(N_CORES)],
                              axis=0)
        # [B, 2, 128, 784] f16 (d-major, col-major pixels) -> [B, H, W, D]
        arr = full.astype(np.float32).reshape(B, 2, 128, W, H)
        return np.ascontiguousarray(arr.transpose(0, 4, 3, 1, 2)).reshape(
            B, H, W, D)

    return unshard("hd_out"), unshard("st_out")
